# revision 73
# baseline (speedup 1.0000x reference)
"""CGCNN (3x CGConv + global mean pool + MLP) on 8 Trainium2 NeuronCores, v3.

Structure (edge/graph partition parallelism, host does indexing only):
  - Nodes sharded into 8 contiguous ranges; each core owns all in-edges of
    its nodes, so conv segment-sums are core-local.  Within a core, nodes
    are sorted by in-degree (desc) and packed [block=128 nodes] x [dg slot
    columns]; consecutive blocks share one padded dg (multiple of 4) in
    variable-size "chunks" planned by a small DP (slots + per-chunk fixed
    cost), capped at CAP slot-columns per partition.
  - Per conv layer the host gathers a 12-lane bf16 payload per slot:
    [T~[src](10), e, e] (pure indexing from the device-computed prescaled
    T table; e duplicated so the broadcast-over-lanes add has a contiguous
    pair in the last AP dim and runs in DVE 2x mode).  Device per chunk:
    z~ = pay.T + A~[dst] (DVE 2x) + e (DVE 2x via the pair trick), then
    in-place per-lane ACT sigmoid/exp with scale=we, one batched Ln
    (softplus tail), msg mult + two halving adds + a short reduce (DVE),
    x' = x + agg/deg (Pool), and the next layer's tables via STT chains on
    DVE with the bias term on ACT (overlapped with conv chunks).  A few
    trailing A-add lanes ride Pool (per-launch KNOBS) to balance engines.
  - The initial table launch runs on the otherwise-idle PE: the host feeds
    x0 feature-major on 30 partitions plus a block-diagonal weight matrix,
    one bf16 matmul per 512-column stripe lands in a PSUM bank, and an ACT
    Identity pass adds per-lane biases; both layouts are host-reindexed.
  - Pooling runs as a final tiny core-local launch on a graph-aligned node
    relayout: per-128-block partial sums via ones-matmuls on PE, a reduce,
    inverse-count scaling, and each core's own 64-graph MLP; the host
    concatenates the per-core outputs (no collective needed).
"""
import sys

sys.path.insert(0, "/opt/trn_rl_repo")

import numpy as np
import ml_dtypes

from concourse import bass, mybir, tile
from concourse.bass_utils import run_bass_kernel_spmd

# ---------------------------------------------------------------------------
# Workarounds for the installed walrus's per-instruction sync-wait cap (same
# as the v1 kernel): split excess waits into standalone EventSemaphore
# instructions and replace the TileContext-exit drain.
# ---------------------------------------------------------------------------
_MAX_WAITS = 1
_wsplit_counter = [0]


def _split_excess_waits(ordered):
    for insts in ordered.values():
        out = []
        for inst in insts:
            si = inst.sync_info
            waits = list(si.on_wait) if si is not None and si.on_wait else []
            if len(waits) > _MAX_WAITS:
                keep = waits[-_MAX_WAITS:]
                for w in waits[:-_MAX_WAITS]:
                    _wsplit_counter[0] += 1
                    wi = mybir.InstEventSemaphore(
                        name=f"WSPLIT-{_wsplit_counter[0]}", ins=[], outs=[]
                    )
                    wi.engine = inst.engine
                    wi.sync_info = mybir.SyncInfo(on_wait=[w], on_update=[])
                    if inst.debug is not None:
                        wi.debug = inst.debug
                    out.append(wi)
                inst.sync_info = mybir.SyncInfo(
                    on_wait=keep,
                    on_update=list(si.on_update) if si.on_update else [],
                )
            out.append(inst)
        insts[:] = out


_orig_lower = tile.TileContext._lower_ordered_insts


def _patched_lower(self, ordered):
    _split_excess_waits(ordered)
    return _orig_lower(self, ordered)


def _patched_drain_and_barrier(self, tick_clock, wait_clock):
    nc = self.nc
    nc.all_engine_barrier()
    nc.sync.drain(semaphore_range=nc._kernel_sem_range)
    nc.all_engine_barrier()
    popped = nc._tile_sem_poison_stack.pop()
    assert popped is self._sem_poison
    nc.clear_and_free_semaphores(list(self.sems.allocated().values()))
    nc.all_engine_barrier()


if getattr(tile.TileContext, "_cgcnn_patched", False) is False:
    tile.TileContext._lower_ordered_insts = _patched_lower
    tile.TileContext._drain_and_barrier = _patched_drain_and_barrier
    tile.TileContext._cgcnn_patched = True

BF16 = mybir.dt.bfloat16
F32 = mybir.dt.float32
NPBF16 = ml_dtypes.bfloat16

N_NODES = 500_000
N_EDGES = 16_000_000
IN_DIM = 5
HIDDEN = 64
N_GRAPHS = 512
NCORES = 8
NPC = N_NODES // NCORES          # 62500 nodes per core
P = 128
NBLK = 504                       # 504*128 = 64512 >= 62500
NPAD = NBLK * P
PAD_T = -10000.0                 # T-table pad row -> z << 0 -> msg == 0
CAP = 1024                       # max slot-columns per partition per chunk
ROUND = 4                        # dg padded to multiple of 4 (2 halvings)
LANES = 12                       # 10 T~ lanes + duplicated e pair
CHUNK_FIXED = 80                 # DP chunk fixed cost, in slot-column units
NSLAB = 6                        # table-build slabs per conv launch
TGRP = 12                        # node blocks per PE table-build group

# schedule-shape knobs (tuned against TimelineSim)
KNOBS = dict(
    act_k0=True,      # table-build k==0 term on ACT (else STT on tb engine)
    tb_dve=20,        # table-build STT lanes on DVE (Pool STT is not a
                      # legal ISA op - TensorScalarPtr fails engine check)
    halv_pool=0,      # how many of the 2 halving adds run on Pool (0/1/2)
    slab_dma="slab",  # "slab": per-slab out DMAs on SP; "end": single at end
    xupd_pool=True,   # x-update TT pair on Pool (else DVE)
    hp_adds=False,    # high_priority on the z-assembly adds
    apool=2,          # trailing lanes of the sp-stream A-add on Pool (B1/B2)
    apool3=2,         # same for the last conv (no table build)
    apool2=0,         # trailing lanes of the sigma-stream A-add on Pool
    epool=2,          # trailing lanes of the sp-stream e-add on Pool
    epool3=2,         # same for the last conv
)
GPC = N_GRAPHS // NCORES         # 64 graphs per core in the pooling launch

_cache = {}


# ---------------------------------------------------------------------------
# weight packing: table-build coefficients
# ---------------------------------------------------------------------------
def _build_wpacks(params):
    """Per-layer coefficient packs.

    t20 table lanes: 0..9 = T (gathered by src), 10..19 = A (dst-local).
      T_c[n] = sum_k x[n,k] * tc[k,c]          (c in 0..10)
      A_c[n] = sum_k x[n,k] * ac[k,c] + ab[c]
      z_c    = T_c[src] + A_c[dst] + e*we[c]
    with c 0..5 the sigmoid (Wf) half and 5..10 the softplus (Ws) half.
    """
    packs = []
    for l in (1, 2, 3):
        Wf, bf = params[f"Wf{l}"], params[f"bf{l}"]
        Ws, bs = params[f"Ws{l}"], params[f"bs{l}"]
        tc = np.zeros((IN_DIM, 10), np.float32)
        ac = np.zeros((IN_DIM, 10), np.float32)
        for k in range(IN_DIM):
            tc[k, :IN_DIM] = Wf[:, IN_DIM + k]
            tc[k, IN_DIM:] = Ws[:, IN_DIM + k]
            ac[k, :IN_DIM] = Wf[:, k]
            ac[k, IN_DIM:] = Ws[:, k]
        ab = np.concatenate([bf, bs]).astype(np.float32)
        we = np.concatenate([Wf[:, 2 * IN_DIM], Ws[:, 2 * IN_DIM]]).astype(np.float32)
        # we-rescale: tables store T/we and A/we so z = we*(T~+A~+e); the
        # scale we_c rides the per-lane ACT instruction.  Error-neutral in
        # floating point (one rounding of the scaled sum, scaled back).
        tco = np.zeros((20, IN_DIM), np.float32)
        tbias = np.zeros((20,), np.float32)
        tco[0:10] = tc.T / we[:, None]
        tco[10:20] = ac.T / we[:, None]
        tbias[10:20] = ab / we
        padrow = (-1e8 * np.sign(we)).astype(np.float32)
        packs.append(dict(tco=tco.reshape(-1), tbias=tbias, we=we, padrow=padrow))
    return packs


# ---------------------------------------------------------------------------
# chunk planning + per-core static edge layout (indexing only)
# ---------------------------------------------------------------------------
def _plan_chunks(db):
    """db: per-block unified max in-degree (sorted desc).  DP over block
    boundaries minimizing padded slots + per-chunk fixed cost; each chunk
    (b0, nb, dgc) has uniform dgc (multiple of ROUND), nb*dgc <= CAP."""
    nblk = len(db)
    n = nblk
    while n > 0 and db[n - 1] <= 0:
        n -= 1
    dgr = [int(-(-int(db[i]) // ROUND) * ROUND) for i in range(n)]
    INF = float("inf")
    best = [INF] * (n + 1)
    prev = [0] * (n + 1)
    best[n] = 0.0
    for i in range(n - 1, -1, -1):
        dgc = dgr[i]
        jmax = min(n, i + max(1, CAP // dgc))
        for j in range(i + 1, jmax + 1):
            c = (j - i) * dgc + CHUNK_FIXED + best[j]
            if c < best[i]:
                best[i] = c
                prev[i] = j
    chunks = []
    i = 0
    while i < n:
        j = prev[i]
        chunks.append((i, j - i, dgr[i]))
        i = j
    # split the final chunk into <=384-col pieces: the last chunk's serial
    # chain (adds -> ACT -> mult -> halvings -> reduce) is pure tail latency
    b0, nb, dgc = chunks[-1]
    if nb * dgc > 384:
        step = max(1, 384 // dgc)
        chunks.pop()
        while nb > 0:
            take = min(step, nb)
            chunks.append((b0, take, dgc))
            b0 += take
            nb -= take
    return chunks


def _preprocess(edge_index, edge_attr, batch):
    src = np.asarray(edge_index[0], dtype=np.int64)
    dst = np.asarray(edge_index[1], dtype=np.int64)
    attr = np.asarray(edge_attr, dtype=np.float32).reshape(-1)
    batch = np.asarray(batch, dtype=np.int64)

    deg = np.bincount(dst, minlength=N_NODES)
    eorder = np.argsort(dst, kind="stable")
    ssrc = src[eorder].astype(np.int32)
    sattr = attr[eorder].astype(NPBF16)
    starts = np.zeros(N_NODES + 1, np.int64)
    np.cumsum(deg, out=starts[1:])

    # per-core degree-sorted node layout
    node_arrs, degs_blocks = [], []
    for c in range(NCORES):
        lo = c * NPC
        degs = deg[lo : lo + NPC]
        order = np.argsort(-degs, kind="stable")
        node_arr = np.full(NPAD, -1, np.int64)
        node_arr[:NPC] = lo + order
        node_arr = node_arr.reshape(NBLK, P)
        node_arrs.append(node_arr)
        nd = np.where(node_arr >= 0, deg[np.clip(node_arr, 0, None)], 0)
        degs_blocks.append(nd.max(axis=1))

    db_u = np.stack(degs_blocks).max(axis=0)  # unified across cores
    chunks = _plan_chunks(db_u)
    TOT = sum(nb * dgc for _, nb, dgc in chunks)

    cores = []
    for c in range(NCORES):
        node_arr = node_arrs[c]
        nd = np.where(node_arr >= 0, deg[np.clip(node_arr, 0, None)], 0)
        slotsrc = np.full((P, TOT), N_NODES, np.int32)
        slotattr = np.zeros((P, TOT), NPBF16)
        off = 0
        for b0, nb, dgc in chunks:
            nbv = node_arr[b0 : b0 + nb]              # [nb, P]
            dbv = nd[b0 : b0 + nb]                    # [nb, P]
            j = np.arange(dgc)
            eid = starts[np.clip(nbv, 0, None)][:, :, None] + j   # [nb,P,dgc]
            ok = j[None, None, :] < dbv[:, :, None]
            sg = np.where(ok, ssrc[np.clip(eid, 0, N_EDGES - 1)], N_NODES)
            ag = np.where(ok, sattr[np.clip(eid, 0, N_EDGES - 1)], NPBF16(0.0))
            # device order: [P][b][j]
            n = nb * dgc
            slotsrc[:, off : off + n] = sg.transpose(1, 0, 2).reshape(P, n)
            slotattr[:, off : off + n] = ag.transpose(1, 0, 2).reshape(P, n)
            off += n
        invdeg = (1.0 / np.maximum(nd, 1)).astype(np.float32)     # [NBLK, P]
        invdeg = np.ascontiguousarray(invdeg.T)                   # [P, NBLK]
        cores.append(
            dict(node_arr=node_arr, slotsrc=slotsrc, slotattr=slotattr,
                 invdeg=invdeg)
        )

    cnt = np.bincount(batch, minlength=N_GRAPHS).astype(np.float32)
    invcnt = (1.0 / np.maximum(cnt, 1.0)).astype(np.float32)
    return cores, chunks, TOT, cnt, invcnt


def _pool_layout(cnt):
    """Graph-aligned pooling layout: core c owns graphs [64c, 64c+64), every
    graph padded to the same NBG whole 128-node blocks (uniform structure so
    all cores share one program).  Returns per-core node_pool_arr
    [GPC*NBG, P] plus NBG."""
    gstart = np.zeros(N_GRAPHS + 1, np.int64)
    np.cumsum(cnt.astype(np.int64), out=gstart[1:])
    NBG = int(-(-cnt.max() // P))
    per_core = []
    for c in range(NCORES):
        node_pool_arr = np.full((GPC * NBG, P), -1, np.int64)
        for gl in range(GPC):
            g = c * GPC + gl
            n0, n1 = gstart[g], gstart[g + 1]
            nn = int(n1 - n0)
            ids = node_pool_arr[gl * NBG : (gl + 1) * NBG].reshape(-1)
            ids[:nn] = np.arange(n0, n1)
        per_core.append(node_pool_arr)
    return per_core, NBG


def _node_layout(arr_full, node_arr, width, dtype=np.float32):
    """[N_NODES, width] -> [P, NBLK*width] in the given node layout."""
    nblk = node_arr.shape[0]
    out = np.zeros((nblk, P, width), dtype)
    v = node_arr >= 0
    out[v] = arr_full[node_arr[v]]
    return np.ascontiguousarray(out.transpose(1, 0, 2).reshape(P, nblk * width))


# ---------------------------------------------------------------------------
# device programs
# ---------------------------------------------------------------------------
def _table_build(nc, xfm_t, tco_t, tbias_t, t20_t, s0, s1, dve_lanes=20,
                 act_k0=False):
    """t20[c, s0:s1] = sum_k xfm[k, s0:s1]*tco[c*5+k] + tbias[c].  The k==0
    term rides the ACT engine (Identity, scale+bias are per-partition
    scalars) when act_k0 is set; the remaining MACs per lane are STT chains
    on DVE/Pool.  Called per block-slab so it pipelines behind the chunks."""
    w = s1 - s0
    for c in range(10, 20):
        eng = nc.vector if c < dve_lanes else nc.gpsimd
        if act_k0:
            nc.scalar.activation(
                out=t20_t[:, s0:s1, c - 10],
                in_=xfm_t[:, 0, s0:s1],
                func=mybir.ActivationFunctionType.Identity,
                scale=tco_t[:, c * IN_DIM : c * IN_DIM + 1],
                bias=tbias_t[:, c : c + 1],
            )
        for k in range(0 if not act_k0 else 1, IN_DIM):
            in1 = (
                tbias_t[:, c : c + 1].to_broadcast([P, w])
                if k == 0
                else t20_t[:, s0:s1, c - 10]
            )
            eng.scalar_tensor_tensor(
                out=t20_t[:, s0:s1, c - 10],
                in0=xfm_t[:, k, s0:s1],
                scalar=tco_t[:, c * IN_DIM + k : c * IN_DIM + k + 1],
                in1=in1,
                op0=mybir.AluOpType.mult,
                op1=mybir.AluOpType.add,
            )


def _xfm_copy(nc, res, x_t, s0, s1):
    """feature-major bf16 copy of x[:, s0:s1, :] into a shared xfm tile."""
    if not hasattr(nc, "_xfm_t"):
        nc._xfm_t = res.tile([P, IN_DIM, NBLK], BF16)
    xfm_t = nc._xfm_t
    xsrc = bass.AP(
        x_t[:].tensor,
        x_t[:].offset + s0 * IN_DIM,
        [x_t[:].ap[0], (1, IN_DIM), (IN_DIM, s1 - s0)],
    )
    nc.vector.tensor_copy(out=xfm_t[:, :, s0:s1], in_=xsrc)
    return xfm_t


def _build_conv(chunks, TOT, last):
    """One conv layer.  last=True: skip next-table build."""
    nc = bass.Bass("TRN2", target_bir_lowering=False, debug=False,
                   num_devices=NCORES)
    pay_d = nc.dram_tensor("pay", [P, TOT * LANES], BF16, kind="ExternalInput").ap()
    x_d = nc.dram_tensor("x", [P, NBLK * IN_DIM], F32, kind="ExternalInput").ap()
    a_d = nc.dram_tensor("atbl", [P, NBLK * 10], BF16, kind="ExternalInput").ap()
    iv_d = nc.dram_tensor("invdeg", [P, NBLK], F32, kind="ExternalInput").ap()
    we_d = nc.dram_tensor("wesc", [P, 10], F32, kind="ExternalInput").ap()
    if not last:
        tco_d = nc.dram_tensor("tco", [P, 100], F32, kind="ExternalInput").ap()
        tb_d = nc.dram_tensor("tbias", [P, 20], F32, kind="ExternalInput").ap()
        wt_d = nc.dram_tensor("wt10", [TGRP * IN_DIM, TGRP * 10], BF16,
                              kind="ExternalInput").ap()
        id_d = nc.dram_tensor("id128", [P, P], F32, kind="ExternalInput").ap()
        t20a_d = nc.dram_tensor("t20a", [P, 10 * NBLK], BF16,
                                kind="ExternalOutput").ap()
        t20t_d = nc.dram_tensor("t20t", [TGRP * 10, (NBLK // TGRP) * P], BF16,
                                kind="ExternalOutput").ap()
    xn_d = nc.dram_tensor("xn", [P, NBLK * IN_DIM], F32, kind="ExternalOutput").ap()

    with tile.TileContext(nc) as tc:
        with (
            tc.tile_pool(name="res", bufs=1) as res,
            tc.tile_pool(name="work", bufs=5) as work,
            tc.tile_pool(name="work2", bufs=3) as work2,
            tc.tile_pool(name="psum", bufs=4, space="PSUM") as psum,
        ):
            x_t = res.tile([P, NBLK, IN_DIM], F32)
            a_t = res.tile([P, NBLK, 10], BF16)
            iv_t = res.tile([P, NBLK], F32)
            we_t = res.tile([P, 10], F32)
            nc.sync.dma_start(out=x_t[:], in_=x_d[:, :])
            nc.sync.dma_start(out=a_t[:], in_=a_d[:, :])
            nc.sync.dma_start(out=iv_t[:], in_=iv_d[:, :])
            nc.sync.dma_start(out=we_t[:], in_=we_d[:, :])
            if not last:
                tco_t = res.tile([P, 100], F32)
                tb_t = res.tile([P, 20], F32)
                wt_t = res.tile([TGRP * IN_DIM, TGRP * 10], BF16)
                id_t = res.tile([P, P], F32)
                nc.sync.dma_start(out=tco_t[:], in_=tco_d[:, :])
                nc.sync.dma_start(out=tb_t[:], in_=tb_d[:, :])
                nc.sync.dma_start(out=wt_t[:], in_=wt_d[:, :])
                nc.sync.dma_start(out=id_t[:], in_=id_d[:, :])

            if not last:
                t20_t = res.tile([P, NBLK, 10], BF16)
                t20t_t = res.tile([TGRP * 10, (NBLK // TGRP) * P], BF16)
            slab_edges = [(s + 1) * NBLK // NSLAB for s in range(NSLAB)]
            slab_done = 0
            soff = 0
            def head(b0, nb, dgc, soff):
                """DMA + z-assembly for one chunk; returns the pay tile."""
                n = nb * dgc
                pay_t = work.tile([P, nb, dgc, LANES], BF16, tag="pay")
                src_ap = bass.AP(
                    pay_d.tensor, soff * LANES, [pay_d.ap[0], (1, n * LANES)]
                )
                nc.sync.dma_start(out=pay_t[:], in_=src_ap)
                pb = pay_t[:]
                row = (LANES * dgc, nb)

                def lanes(off, shape):
                    return bass.AP(pb.tensor, pb.offset + off,
                                   [pb.ap[0], row] + shape)

                # z~ = pay.T~ + A~[dst] + e, both adds DVE 2x (the e add via
                # the duplicated-pair lanes 10:12 so its broadcast view
                # keeps a contiguous pair in the last AP dim).  Split into
                # two lane-group streams (4:10 then 0:4) so the ACT exp/Ln
                # chain starts after half the assembly and overlaps the rest.
                def zassy(c0, c1, ap, ep_pool):
                    """A-add (split DVE/Pool at lane c1-ap) then e-pair add
                    (trailing ep_pool lanes on Pool)."""
                    cs = c1 - ap
                    for e0, e1, eng in ((c0, cs, nc.vector),
                                        (cs, c1, nc.gpsimd)):
                        if e0 == e1:
                            continue
                        zg = lanes(e0, [(LANES, dgc), (1, e1 - e0)])
                        eng.tensor_tensor(
                            out=zg, in0=zg,
                            in1=a_t[:, b0 : b0 + nb, None, e0:e1].to_broadcast(
                                [P, nb, dgc, e1 - e0]
                            ),
                            op=mybir.AluOpType.add,
                        )
                    es = c1 - ep_pool
                    for e0, e1, eng in ((c0, es, nc.vector),
                                        (es, c1, nc.gpsimd)):
                        if e0 == e1:
                            continue
                        zp = lanes(e0, [(LANES, dgc),
                                        (2, (e1 - e0) // 2), (1, 2)])
                        epv = lanes(10, [(LANES, dgc),
                                         (0, (e1 - e0) // 2), (1, 2)])
                        eng.tensor_tensor(
                            out=zp, in0=zp, in1=epv, op=mybir.AluOpType.add
                        )

                ap = KNOBS["apool3"] if last else KNOBS["apool"]
                epl = KNOBS["epool3"] if last else KNOBS["epool"]
                zassy(4, 10, ap, epl)
                zassy(0, 4, KNOBS["apool2"], 0)
                return pay_t, lanes

            def tail(b0, nb, dgc, pay_t, lanes, is_final):
                nonlocal slab_done
                # in-place per-lane ACT: exp lanes first (feeds the batched
                # Ln), then sigmoid lanes; scale=we_c rides each pass
                for c in (5, 6, 7, 8, 9, 4, 0, 1, 2, 3):
                    fn = (
                        mybir.ActivationFunctionType.Sigmoid
                        if c < IN_DIM
                        else mybir.ActivationFunctionType.Exp
                    )
                    lv = lanes(c, [(LANES, dgc)])
                    nc.scalar.activation(
                        out=lv, in_=lv, func=fn, scale=we_t[:, c : c + 1],
                    )
                sp5 = lanes(5, [(LANES, dgc), (1, IN_DIM)])
                nc.scalar.activation(
                    out=sp5, in_=sp5,
                    func=mybir.ActivationFunctionType.Ln, bias=1.0,
                )
                # msg = sg*sp into a separate slot-major tile (DVE 2x);
                # frees the pay slot for the next prefetch 3 stages earlier
                msg_t = work2.tile([P, nb, dgc, IN_DIM], BF16, tag="msg")
                nc.vector.tensor_tensor(
                    out=msg_t[:],
                    in0=lanes(0, [(LANES, dgc), (1, IN_DIM)]),
                    in1=lanes(5, [(LANES, dgc), (1, IN_DIM)]),
                    op=mybir.AluOpType.mult,
                )
                # two halving adds (Pool/DVE per knob), then a short DVE
                # reduce over the remaining quarter (dgc innermost)
                h = dgc // 2
                q = dgc // 4
                e1 = nc.gpsimd if KNOBS["halv_pool"] >= 1 else nc.vector
                e2 = nc.gpsimd if KNOBS["halv_pool"] >= 2 else nc.vector
                e1.tensor_tensor(
                    out=msg_t[:, :, 0:h, :],
                    in0=msg_t[:, :, 0:h, :],
                    in1=msg_t[:, :, h:dgc, :],
                    op=mybir.AluOpType.add,
                )
                e2.tensor_tensor(
                    out=msg_t[:, :, 0:q, :],
                    in0=msg_t[:, :, 0:q, :],
                    in1=msg_t[:, :, q:h, :],
                    op=mybir.AluOpType.add,
                )
                agg_t = work2.tile([P, nb, IN_DIM], F32, tag="agg")
                mb = msg_t[:]
                nc.vector.tensor_reduce(
                    out=agg_t[:],
                    in_=bass.AP(
                        mb.tensor, mb.offset,
                        [mb.ap[0], (dgc * IN_DIM, nb),
                         (1, IN_DIM), (IN_DIM, q)],
                    ),
                    axis=mybir.AxisListType.X,
                    op=mybir.AluOpType.add,
                )
                # x' = x + agg*invdeg (f32, small)
                exu = nc.gpsimd if KNOBS["xupd_pool"] else nc.vector
                exu.tensor_tensor(
                    out=agg_t[:],
                    in0=agg_t[:],
                    in1=iv_t[:, b0 : b0 + nb, None].to_broadcast(
                        [P, nb, IN_DIM]
                    ),
                    op=mybir.AluOpType.mult,
                )
                exu.tensor_tensor(
                    out=x_t[:, b0 : b0 + nb, :],
                    in0=agg_t[:],
                    in1=x_t[:, b0 : b0 + nb, :],
                    op=mybir.AluOpType.add,
                )
                while slab_done < len(slab_edges) and (
                    is_final or b0 + nb >= slab_edges[slab_done]
                ):
                    s0 = slab_edges[slab_done - 1] if slab_done else 0
                    s1 = slab_edges[slab_done]
                    if not last:
                        # A-lanes: STT chains on DVE (node-partition layout)
                        xfm_t = _xfm_copy(nc, res, x_t, s0, s1)
                        _table_build(nc, xfm_t, tco_t, tb_t, t20_t, s0, s1,
                                     dve_lanes=KNOBS["tb_dve"],
                                     act_k0=KNOBS["act_k0"])
                        # T-lanes on the idle PE: transpose 12-block x
                        # groups to PSUM, bf16 them, block-diag matmul,
                        # ACT-identity cast into the (lane, node) table
                        for g in range(s0 // TGRP, s1 // TGRP):
                            xt_ps = psum.tile([TGRP * IN_DIM, P], F32,
                                              tag="xt")
                            nc.tensor.transpose(
                                out=xt_ps[:],
                                in_=x_t[:, g * TGRP : (g + 1) * TGRP, :],
                                identity=id_t[:],
                            )
                            xt_sb = work2.tile([TGRP * IN_DIM, P], BF16,
                                               tag="xts")
                            nc.vector.tensor_copy(out=xt_sb[:], in_=xt_ps[:])
                            tt_ps = psum.tile([TGRP * 10, P], F32, tag="tt")
                            nc.tensor.matmul(
                                out=tt_ps[:], lhsT=wt_t[:], rhs=xt_sb[:],
                                start=True, stop=True,
                            )
                            nc.scalar.activation(
                                out=t20t_t[:, g * P : (g + 1) * P],
                                in_=tt_ps[:],
                                func=mybir.ActivationFunctionType.Identity,
                            )
                    if KNOBS["slab_dma"] == "slab":
                        if not last:
                            nc.sync.dma_start(
                                out=t20a_d[:, s0 * 10 : s1 * 10],
                                in_=t20_t[:, s0:s1, :],
                            )
                            nc.sync.dma_start(
                                out=t20t_d[:, (s0 // TGRP) * P
                                           : (s1 // TGRP) * P],
                                in_=t20t_t[:, (s0 // TGRP) * P
                                           : (s1 // TGRP) * P],
                            )
                        nc.sync.dma_start(
                            out=xn_d[:, s0 * IN_DIM : s1 * IN_DIM],
                            in_=x_t[:, s0:s1, :],
                        )
                    slab_done += 1

            # 1-deep software pipeline: emit chunk k+1's DMA + z-assembly
            # before chunk k's ACT/tail so DVE always feeds ACT first
            pend = None
            for b0, nb, dgc in chunks:
                ht = head(b0, nb, dgc, soff)
                soff += nb * dgc
                if pend is not None:
                    tail(*pend, is_final=False)
                pend = (b0, nb, dgc, *ht)
            tail(*pend, is_final=True)
            if KNOBS["slab_dma"] == "end":
                if not last:
                    nc.sync.dma_start(out=t20a_d[:, :], in_=t20_t[:])
                    nc.sync.dma_start(out=t20t_d[:, :], in_=t20t_t[:])
                nc.sync.dma_start(out=xn_d[:, :], in_=x_t[:])
    return nc


G6 = 6                            # nodes per PE group in launch A
NC6 = NPAD // G6                  # 10752 columns
AW = 512                          # columns per matmul (one PSUM bank)


def _build_A():
    """Initial tables from x0, computed on the idle PE.

    The host feeds x0 feature-major on 30 partitions ((group, feat) rows,
    NC6 node-columns) plus a block-diagonal [30, 120] weight matrix, and
    reads t20 back as [120 = (group, lane), NC6] -- both layouts are
    host-(re)indexed, so the node-partition constraint of the conv
    launches does not apply here.  Per 512-column stripe: one bf16 matmul
    into a PSUM bank, then one ACT Identity pass adds the per-lane bias
    and casts to bf16."""
    nc = bass.Bass("TRN2", target_bir_lowering=False, debug=False,
                   num_devices=NCORES)
    xt_d = nc.dram_tensor("xt30", [G6 * IN_DIM, NC6], BF16,
                          kind="ExternalInput").ap()
    w_d = nc.dram_tensor("wblk", [G6 * IN_DIM, G6 * 20], BF16,
                         kind="ExternalInput").ap()
    bias_d = nc.dram_tensor("bias120", [G6 * 20, 1], F32,
                            kind="ExternalInput").ap()
    t20_d = nc.dram_tensor("t20p", [G6 * 20, NC6], BF16,
                           kind="ExternalOutput").ap()
    with tile.TileContext(nc) as tc:
        with (
            tc.tile_pool(name="res", bufs=1) as res,
            tc.tile_pool(name="psum", bufs=4, space="PSUM") as psum,
        ):
            xt_t = res.tile([G6 * IN_DIM, NC6], BF16)
            w_t = res.tile([G6 * IN_DIM, G6 * 20], BF16)
            bias_t = res.tile([G6 * 20, 1], F32)
            nc.sync.dma_start(out=xt_t[:], in_=xt_d[:, :])
            nc.sync.dma_start(out=w_t[:], in_=w_d[:, :])
            nc.sync.dma_start(out=bias_t[:], in_=bias_d[:, :])
            t20_t = res.tile([G6 * 20, NC6], BF16)
            for j0 in range(0, NC6, AW):
                ps = psum.tile([G6 * 20, AW], F32, tag="ps")
                nc.tensor.matmul(
                    out=ps[:], lhsT=w_t[:], rhs=xt_t[:, j0 : j0 + AW],
                    start=True, stop=True,
                )
                nc.scalar.activation(
                    out=t20_t[:, j0 : j0 + AW], in_=ps[:],
                    func=mybir.ActivationFunctionType.Identity,
                    bias=bias_t[:, 0:1],
                )
            nc.sync.dma_start(out=t20_d[:, :], in_=t20_t[:])
    return nc


def _build_P(NBG):
    """Pooling + MLP launch, fully core-local (no collective).  xp is
    [P, GPC*NBG*5] graph-aligned (NBG blocks per graph) holding only this
    core's GPC graphs; per-block partial sums land in one flat row via
    ones-matmuls, a reduce folds the NBG blocks, inverse counts scale the
    sums, a DRAM bounce lands pooled[c5, g] on 5 partitions, and the core
    runs the tiny MLP for its own GPC graphs (host concatenates cores)."""
    NPBLK = GPC * NBG
    GCH = 8                          # graphs per stage-1 matmul (360 cols)
    nc = bass.Bass("TRN2", target_bir_lowering=False, debug=False,
                   num_devices=NCORES)
    xp_d = nc.dram_tensor("xp", [P, NPBLK * IN_DIM], F32,
                          kind="ExternalInput").ap()
    icm_d = nc.dram_tensor("invcntm", [1, GPC * IN_DIM], F32,
                           kind="ExternalInput").ap()
    w1t_d = nc.dram_tensor("w1t", [IN_DIM, HIDDEN], F32, kind="ExternalInput").ap()
    b1_d = nc.dram_tensor("b1", [HIDDEN, 1], F32, kind="ExternalInput").ap()
    w2t_d = nc.dram_tensor("w2t", [HIDDEN, 1], F32, kind="ExternalInput").ap()
    b2_d = nc.dram_tensor("b2", [1, 1], F32, kind="ExternalInput").ap()
    out_d = nc.dram_tensor("out", [1, GPC], F32, kind="ExternalOutput").ap()

    with tile.TileContext(nc) as tc:
        with (
            tc.tile_pool(name="res", bufs=1) as res,
            tc.tile_pool(name="psum", bufs=2, space="PSUM") as psum,
        ):
            xp_t = res.tile([P, NPBLK, IN_DIM], F32)
            icm_t = res.tile([1, GPC * IN_DIM], F32)
            nc.sync.dma_start(out=xp_t[:], in_=xp_d[:, :])
            nc.sync.dma_start(out=icm_t[:], in_=icm_d[:, :])
            w1t_t = res.tile([IN_DIM, HIDDEN], F32)
            b1_t = res.tile([HIDDEN, 1], F32)
            w2t_t = res.tile([HIDDEN, 1], F32)
            b2_t = res.tile([1, 1], F32)
            nc.sync.dma_start(out=w1t_t[:], in_=w1t_d[:, :])
            nc.sync.dma_start(out=b1_t[:], in_=b1_d[:, :])
            nc.sync.dma_start(out=w2t_t[:], in_=w2t_d[:, :])
            nc.sync.dma_start(out=b2_t[:], in_=b2_d[:, :])
            ones_t = res.tile([P, 1], F32)
            nc.vector.memset(ones_t[:], 1.0)
            # stage 1: per-block sums -> flat row, order (g, c5, b)
            bs_row = res.tile([1, GPC, IN_DIM, NBG], F32)
            for gi in range(GPC // GCH):
                rhs = bass.AP(
                    xp_t[:].tensor,
                    xp_t[:].offset + gi * GCH * NBG * IN_DIM,
                    [
                        xp_t[:].ap[0],
                        (NBG * IN_DIM, GCH),
                        (1, IN_DIM),
                        (IN_DIM, NBG),
                    ],
                )
                bs_ps = psum.tile([1, GCH, IN_DIM, NBG], F32, tag="bs")
                nc.tensor.matmul(
                    out=bs_ps[:], lhsT=ones_t[:], rhs=rhs,
                    start=True, stop=True,
                )
                nc.vector.tensor_copy(
                    out=bs_row[:, gi * GCH : (gi + 1) * GCH, :, :],
                    in_=bs_ps[:],
                )
            # fold the NBG blocks of each (g, c5), scale by 1/cnt (icm fed
            # in the same flat (g, c5) order)
            pl_row = res.tile([1, GPC, IN_DIM], F32)
            nc.vector.tensor_reduce(
                out=pl_row[:], in_=bs_row[:],
                axis=mybir.AxisListType.X, op=mybir.AluOpType.add,
            )
            nc.vector.tensor_tensor(
                out=pl_row[:],
                in0=bass.AP(
                    pl_row[:].tensor, pl_row[:].offset,
                    [pl_row[:].ap[0], (1, GPC * IN_DIM)],
                ),
                in1=bass.AP(
                    icm_t[:].tensor, icm_t[:].offset,
                    [icm_t[:].ap[0], (1, GPC * IN_DIM)],
                ),
                op=mybir.AluOpType.mult,
            )
            with tc.tile_pool(name="dram", bufs=1, space="DRAM") as dpool:
                cc = dpool.tile([1, GPC * IN_DIM], F32)
                nc.sync.dma_start(out=cc[:], in_=pl_row[:])
                # land flat (g, c5) on 5 partitions: pooled[c5, g]
                pooled_t = res.tile([IN_DIM, GPC], F32)
                csrc = bass.AP(
                    cc[:].tensor,
                    cc[:].offset,
                    [(1, IN_DIM), (IN_DIM, GPC)],
                )
                nc.sync.dma_start(out=pooled_t[:], in_=csrc)
            h_ps = psum.tile([HIDDEN, GPC], F32, tag="h")
            nc.tensor.matmul(
                out=h_ps[:], lhsT=w1t_t[:], rhs=pooled_t[:], start=True, stop=True
            )
            h_t = res.tile([HIDDEN, GPC], F32)
            nc.scalar.activation(
                out=h_t[:], in_=h_ps[:],
                func=mybir.ActivationFunctionType.Relu, bias=b1_t[:],
            )
            o_ps = psum.tile([1, GPC], F32, tag="o")
            nc.tensor.matmul(
                out=o_ps[:], lhsT=w2t_t[:], rhs=h_t[:], start=True, stop=True
            )
            o_t = res.tile([1, GPC], F32)
            nc.vector.tensor_tensor(
                out=o_t[:],
                in0=o_ps[:],
                in1=b2_t[:, 0:1].to_broadcast([1, GPC]),
                op=mybir.AluOpType.add,
            )
            nc.sync.dma_start(out=out_d[:, :], in_=o_t[:])
    return nc


# ---------------------------------------------------------------------------
# host orchestration
# ---------------------------------------------------------------------------
def _timeline_est(progs):
    from concourse.timeline_sim import TimelineSim

    est = []
    for name, prog in progs:
        try:
            ns = TimelineSim(prog).simulate()
        except Exception as e:
            print(f"TimelineSim {name} failed: {e}")
            ns = None
        est.append((name, ns))
    return est


def kernel(x, edge_index, edge_attr, batch, **params):
    import os

    x = np.asarray(x, dtype=np.float32)
    batch_np = np.asarray(batch, dtype=np.int64)
    cores, chunks, TOT, cnt, invcnt = _preprocess(edge_index, edge_attr, batch)
    pool_cores, NBG = _pool_layout(cnt)
    wpacks = _build_wpacks(params)

    key = (tuple(chunks), NBG)
    if key not in _cache:
        _cache[key] = (
            _build_A(),
            _build_conv(chunks, TOT, last=False),
            _build_conv(chunks, TOT, last=True),
            _build_P(NBG),
        )
    nc_a, nc_b, nc_b3, nc_p = _cache[key]

    core_ids = list(range(NCORES))
    rep = np.ones((P, 1), np.float32)

    id128 = np.eye(P, dtype=np.float32)

    def coef_maps(l):
        wp = wpacks[l]
        wt10 = np.zeros((TGRP * IN_DIM, TGRP * 10), NPBF16)
        tcoT = wp["tco"].reshape(20, IN_DIM)[0:10].T.astype(NPBF16)  # [5, 10]
        for g in range(TGRP):
            wt10[g * IN_DIM : (g + 1) * IN_DIM, g * 10 : (g + 1) * 10] = tcoT
        return {
            "tco": np.ascontiguousarray(rep * wp["tco"].reshape(1, -1)),
            "tbias": np.ascontiguousarray(rep * wp["tbias"].reshape(1, -1)),
            "wt10": wt10,
            "id128": id128,
        }

    # ---- A: initial tables (PE matmul on host-transposed x0) ----
    wp0 = wpacks[0]
    wblk = np.zeros((G6 * IN_DIM, G6 * 20), NPBF16)
    bias120 = np.zeros((G6 * 20, 1), np.float32)
    tco0 = wp0["tco"]
    for g in range(G6):
        wblk[g * IN_DIM : (g + 1) * IN_DIM, g * 20 : (g + 1) * 20] = (
            tco0.reshape(20, IN_DIM).T.astype(NPBF16)
        )
        bias120[g * 20 : (g + 1) * 20, 0] = wp0["tbias"]
    in_maps = []
    x_cur = []
    for c in cores:
        xl = _node_layout(x, c["node_arr"], IN_DIM)
        x_cur.append(xl)
        arr = np.zeros((NPAD, IN_DIM), np.float32)
        na = c["node_arr"].reshape(-1)
        v = na >= 0
        arr[v] = x[na[v]]
        xt30 = np.ascontiguousarray(
            arr.reshape(NC6, G6, IN_DIM).transpose(1, 2, 0).reshape(
                G6 * IN_DIM, NC6
            )
        ).astype(NPBF16)
        in_maps.append({"xt30": xt30, "wblk": wblk, "bias120": bias120})
    res_a = run_bass_kernel_spmd(nc_a, in_maps, core_ids=core_ids)
    exec_times = [res_a.exec_time_ns]

    def assemble_tables(res, padrow):
        """t20a [P, 10*NBLK] (A-lanes, node-partition) + t20t
        [TGRP*10, (NBLK/TGRP)*P] (T-lanes, PE (block12, lane) x (group,
        node) layout) -> (t_full [N+1, 10] bf16, per-core a_in)."""
        t_full = np.empty((N_NODES + 1, 10), NPBF16)
        t_full[N_NODES] = padrow.astype(NPBF16)
        a_ins = []
        ngrp = NBLK // TGRP
        for ci, c in enumerate(cores):
            t20t = np.asarray(res.results[ci]["t20t"])
            # [(b12, c), (g, p)] -> [node(b, p), c] with b = g*TGRP + b12
            tvals = np.ascontiguousarray(
                t20t.reshape(TGRP, 10, ngrp, P).transpose(2, 0, 3, 1)
            ).reshape(NBLK, P, 10)
            na = c["node_arr"]  # [NBLK, P]
            v = na >= 0
            t_full[na[v]] = tvals[v]
            a_ins.append(np.asarray(res.results[ci]["t20a"]))
        return t_full, a_ins

    # assemble layer-1 tables from the PE layout [(g, lane), NC6]
    t_full = np.empty((N_NODES + 1, 10), NPBF16)
    t_full[N_NODES] = wp0["padrow"].astype(NPBF16)
    a_ins = []
    for ci, c in enumerate(cores):
        t20p = np.asarray(res_a.results[ci]["t20p"])
        arr = np.ascontiguousarray(
            t20p.reshape(G6, 20, NC6).transpose(2, 0, 1).reshape(NPAD, 20)
        )
        na = c["node_arr"].reshape(-1)
        v = na >= 0
        t_full[na[v]] = arr[v, 0:10]
        a_ins.append(
            np.ascontiguousarray(
                arr[:, 10:20].reshape(NBLK, P, 10).transpose(1, 0, 2)
            ).reshape(P, NBLK * 10)
        )

    for l in (1, 2, 3):
        last = l == 3
        in_maps = []
        for ci, c in enumerate(cores):
            pay = np.empty((P, TOT, LANES), NPBF16)
            pay[:, :, 0:10] = t_full[c["slotsrc"]]
            pay[:, :, 10] = c["slotattr"]
            pay[:, :, 11] = c["slotattr"]
            m = {
                "pay": pay.reshape(P, TOT * LANES),
                "x": x_cur[ci],
                "atbl": a_ins[ci],
                "invdeg": c["invdeg"],
                "wesc": np.ascontiguousarray(
                    rep * wpacks[l - 1]["we"].reshape(1, -1)
                ),
            }
            if not last:
                m.update(coef_maps(l))
            in_maps.append(m)
        res = run_bass_kernel_spmd(nc_b3 if last else nc_b, in_maps,
                                   core_ids=core_ids)
        exec_times.append(res.exec_time_ns)
        x_cur = [np.asarray(res.results[ci]["xn"]) for ci in range(NCORES)]
        if bool(int(os.environ.get("CGCNN_DUMP", "0"))):
            np.save(f"/tmp/k_x{l}.npy", np.stack(x_cur))
        if not last:
            t_full, a_ins = assemble_tables(res, wpacks[l]["padrow"])

    # ---- P: pooling + MLP ----
    x3_full = np.zeros((N_NODES, IN_DIM), np.float32)
    for ci, c in enumerate(cores):
        na = c["node_arr"]
        v = na >= 0
        xv = x_cur[ci].reshape(P, NBLK, IN_DIM).transpose(1, 0, 2)
        x3_full[na[v]] = xv[v]

    NPBLK = GPC * NBG
    in_maps = []
    for ci in range(NCORES):
        node_pool_arr = pool_cores[ci]
        xp = np.zeros((NPBLK, P, IN_DIM), np.float32)
        v = node_pool_arr >= 0
        xp[v] = x3_full[node_pool_arr[v]]
        icm = np.broadcast_to(
            invcnt[ci * GPC : (ci + 1) * GPC, None], (GPC, IN_DIM)
        )
        in_maps.append(
            {
                "xp": np.ascontiguousarray(
                    xp.transpose(1, 0, 2).reshape(P, NPBLK * IN_DIM)
                ),
                "invcntm": np.ascontiguousarray(icm.reshape(1, -1)),
                "w1t": np.ascontiguousarray(params["W1"].T.astype(np.float32)),
                "b1": np.ascontiguousarray(
                    params["b1"].astype(np.float32).reshape(HIDDEN, 1)
                ),
                "w2t": np.ascontiguousarray(params["W2"].T.astype(np.float32)),
                "b2": np.ascontiguousarray(
                    params["b2"].astype(np.float32).reshape(1, 1)
                ),
            }
        )
    res_p = run_bass_kernel_spmd(nc_p, in_maps, core_ids=core_ids)
    exec_times.append(res_p.exec_time_ns)

    kernel.exec_times = exec_times
    if bool(int(os.environ.get("CGCNN_TRACE", "1"))):
        uniq = _timeline_est(
            [("A", nc_a), ("B", nc_b), ("B3", nc_b3), ("P", nc_p)]
        )
        d = dict(uniq)
        # name->ns pairs per LAUNCH: A, B, B, B3, P
        est = [
            ("A", d["A"]),
            ("B1", d["B"]),
            ("B2", d["B"]),
            ("B3", d["B3"]),
            ("P", d["P"]),
        ]
        kernel.timeline_est = est
        print("cost-model timeline estimate per launch (ns):", est)

    out = np.concatenate(
        [np.asarray(res_p.results[ci]["out"]).reshape(GPC) for ci in range(NCORES)]
    ).reshape(N_GRAPHS, 1)
    return out.astype(np.float32)

